# revision 10
# baseline (speedup 1.0000x reference)
"""HAttentionNetwork Trainium2 kernel (v5).

Strategy (8 NeuronCores, data-parallel over bags):
- 4096 bags are LPT-packed into 80 chunks (10/core, <=64 bags each),
  minimizing max sentences/chunk; sentences are gathered host-side so each
  chunk's sentences are contiguous and padded to Tc tiles of 128.
- Per 128-sentence tile, on device (row-major logits):
    FL[s, 112] = x-tile (two 128-col fp16 stationary blocks, FWL) @ ct
                (PE, PSUM f32; cols = [L0-by-class 53|pad | L1-by-class 53|pad];
                both levels indexed by class so one label onehot serves both)
    E = exp(FL)                                   (ACT, -> fp16 SBUF)
    et[s, l] = sum_c (iota==idx_l) * E-block_l    (DVE scalar_tensor_tensor
                                                   with fused accum_out, x2)
    a2[s, l*64+b] = (iota64==seg_rel) * et_l      (DVE tensor_scalar x2, fp16)
    u2 += a2^T @ [x | 1]                          (PE, segment-sum, PSUM f32)
- Per chunk epilogue: 1/s scale (ACT), PE transpose, disc proj, +bias.
- Issue order is software-pipelined one stage: FL(t+1) goes to the PE queue
  before u2(t), so PE never stalls on the ACT/DVE chain of tile t.
- Per-sentence scalars (label, seg-rel) ride in a small f32 sidecar
  tile (is_equal requires f32 scalar operands).
Numerics: fp16 operands (11-bit mantissa; |logit|<8 so exp<3e3 fits),
fp32 PSUM accumulation everywhere.
DMA/tile: 64KB xt + 64.25KB xh + 1KB sgl (~1034 B/sentence vs 1754
for the f32 baseline).
"""

import numpy as np

N_SENT = 262144
N_BAGS = 4096
HIDDEN = 256
L0 = 14
NCLS = 53
NCORE = 8
CHUNKS_PER_CORE = 10
NCHUNK = NCORE * CHUNKS_PER_CORE
MAX_BAGS_PER_CHUNK = 64
CW = 112  # class cols: [0:53]=L0-by-class (pad 56), [56:109]=L1-by-class (pad 112)

_CACHE = {}


def _patch_tile_drain():
    # This walrus build rejects Drain instructions carrying more than ~1 sync
    # wait. Split the Tile final-drain waits across SP nops, one wait each.
    import concourse.mybir as mybir
    import concourse.tile as tile_mod
    from concourse.vector_clock import ScopedClock

    if getattr(tile_mod.TileContext, "_drain_split_patched", False):
        return

    def _split_drain_and_barrier(self, tick_clock, wait_clock):
        drain_inst = self.nc.sync.drain()
        wait_clock.add_sem_waits(
            drain_inst.ins, ScopedClock({None: tick_clock.global_clock})
        )
        si = drain_inst.ins.sync_info
        waits = list(si.on_wait) if si is not None else []
        if len(waits) > 1:
            drain_inst.ins.sync_info = mybir.SyncInfo(
                on_wait=waits[:1], on_update=list(si.on_update)
            )
            for w in waits[1:]:
                nop = self.nc.sync.nop(nofuse=True, hint="drain_wait_split")
                nop.ins.sync_info = mybir.SyncInfo(on_wait=[w], on_update=[])
        self.nc.all_engine_barrier()
        assert self.sems is not None
        popped = self.nc._tile_sem_poison_stack.pop()
        assert popped is self._sem_poison
        self.nc.clear_and_free_semaphores(list(self.sems.allocated().values()))
        self.nc.all_engine_barrier()

    tile_mod.TileContext._drain_and_barrier = _split_drain_and_barrier
    tile_mod.TileContext._drain_split_patched = True


def _split_all_waits(nc, max_waits=1):
    """This walrus build caps sync-wait commands per instruction very low.
    Move excess waits onto same-engine NOPs inserted just before."""
    import concourse.mybir as mybir

    n = 0
    for f in nc.m.functions:
        for b in f.blocks:
            new = []
            for inst in b.instructions:
                si = getattr(inst, "sync_info", None)
                waits = list(si.on_wait) if si is not None else []
                if len(waits) > max_waits:
                    keep = waits[:max_waits]
                    extra = waits[max_waits:]
                    for w in extra:
                        nop = mybir.InstNoOp(
                            name=f"waitsplit-{n}", ins=[], outs=[]
                        )
                        n += 1
                        nop.engine = inst.engine
                        nop.sync_info = mybir.SyncInfo(
                            on_wait=[w], on_update=[]
                        )
                        new.append(nop)
                    inst.sync_info = mybir.SyncInfo(
                        on_wait=keep, on_update=list(si.on_update)
                    )
                new.append(inst)
            b.instructions[:] = new
    return n


def _segment_ids(scope):
    marks = np.zeros(N_SENT, np.int64)
    np.add.at(marks, scope[1:-1].astype(np.int64), 1)
    return np.cumsum(marks)


def _lpt_chunks(counts):
    """LPT-pack all bags into NCHUNK chunks (<= MAX_BAGS_PER_CHUNK bags
    each), minimizing max sentence count. Returns list of bag-id lists."""
    order = np.argsort(-counts, kind="stable")
    loads = np.zeros(NCHUNK, np.int64)
    nbags = np.zeros(NCHUNK, np.int64)
    assign = [[] for _ in range(NCHUNK)]
    big = np.int64(1 << 60)
    for b in order:
        masked = np.where(nbags < MAX_BAGS_PER_CHUNK, loads, big)
        k = int(np.argmin(masked))
        assign[k].append(int(b))
        loads[k] += int(counts[b])
        nbags[k] += 1
    return assign, int(loads.max())


def _build_bass(Tc):
    import concourse.mybir as mybir
    from concourse import bass
    from concourse.tile import TileContext

    _patch_tile_drain()
    f32 = mybir.dt.float32
    f16 = mybir.dt.float16
    AO = mybir.AluOpType
    G = CHUNKS_PER_CORE * Tc

    nc = bass.Bass("TRN2")
    d_xt = nc.dram_tensor("xt3", [G, 128, 256], f16, kind="ExternalInput")
    d_xh = nc.dram_tensor("xh3", [G, 128, 257], f16, kind="ExternalInput")
    d_sg = nc.dram_tensor("sgl3", [G, 128, 2], f32, kind="ExternalInput")
    d_ct = nc.dram_tensor("ct", [2, 128, CW], f16, kind="ExternalInput")
    d_i56 = nc.dram_tensor("io56", [128, 56], f16, kind="ExternalInput")
    d_i64 = nc.dram_tensor("io64", [128, 64], f16, kind="ExternalInput")
    d_id = nc.dram_tensor("identb", [128, 128], f32, kind="ExternalInput")
    d_dt = nc.dram_tensor("dt4", [128, 212], f16, kind="ExternalInput")
    d_bb = nc.dram_tensor("biasb", [64, 53], f32, kind="ExternalInput")
    d_out = nc.dram_tensor(
        "out", [CHUNKS_PER_CORE, 64, 53], f32, kind="ExternalOutput"
    )

    with TileContext(nc) as tc:
        with (
            tc.tile_pool(name="const", bufs=1) as cpool,
            tc.tile_pool(name="xtp", bufs=6) as xtp,
            tc.tile_pool(name="xhp", bufs=6) as xhp,
            tc.tile_pool(name="sgp", bufs=6) as sgp,
            tc.tile_pool(name="ep", bufs=3) as epool,
            tc.tile_pool(name="jp", bufs=3) as jpool,
            tc.tile_pool(name="etp", bufs=3) as etpool,
            tc.tile_pool(name="a2p", bufs=3) as a2pool,
            tc.tile_pool(name="miscp", bufs=2) as miscp,
            tc.tile_pool(name="ps_fl", bufs=2, space="PSUM") as ps_fl,
            tc.tile_pool(name="ps_u", bufs=2, space="PSUM") as ps_u,
            tc.tile_pool(name="ps_tp", bufs=1, space="PSUM") as ps_tp,
            tc.tile_pool(name="ps_o", bufs=1, space="PSUM") as ps_o,
        ):
            ct0 = cpool.tile([128, CW], f16, tag="ct0")
            ct1 = cpool.tile([128, CW], f16, tag="ct1")
            io56 = cpool.tile([128, 56], f16, tag="io56")
            io64 = cpool.tile([128, 64], f16, tag="io64")
            identb = cpool.tile([128, 128], f32, tag="identb")
            dt4 = cpool.tile([128, 212], f16, tag="dt4")
            bb = cpool.tile([64, 53], f32, tag="bb")
            nc.sync.dma_start(out=ct0[:], in_=d_ct[0])
            nc.sync.dma_start(out=ct1[:], in_=d_ct[1])
            nc.sync.dma_start(out=io56[:], in_=d_i56[:])
            nc.sync.dma_start(out=io64[:], in_=d_i64[:])
            nc.sync.dma_start(out=identb[:], in_=d_id[:])
            nc.sync.dma_start(out=dt4[:], in_=d_dt[:])
            nc.sync.dma_start(out=bb[:], in_=d_bb[:])

            EXP = mybir.ActivationFunctionType.Exp
            CPY = mybir.ActivationFunctionType.Copy

            # one-stage software pipeline: tail(t-1) = [u2 matmul (+chunk
            # epilogue)] is issued after FL(t) so PE stays busy while the
            # ACT/DVE chain of tile t-1 produces a2(t-1).
            pending = [None]

            def epilogue(k, u2):
                seps = miscp.tile([128, 1], f32, tag="seps")
                invs = miscp.tile([128, 1], f32, tag="invs")
                nc.scalar.activation(
                    seps[:], u2[:, 256:257], CPY, bias=1e-30
                )
                nc.vector.reciprocal(invs[:], seps[:])
                repre = miscp.tile([128, 256], f32, tag="repre")
                nc.scalar.activation(
                    repre[:], u2[:, 0:256], CPY, scale=invs[:]
                )
                tp = ps_tp.tile([128, 256], f32, tag="tp")
                nc.tensor.transpose(tp[:, 0:128], repre[:, 0:128], identb[:])
                nc.tensor.transpose(tp[:, 128:256], repre[:, 128:256], identb[:])
                rT = miscp.tile([128, 256], f16, tag="rT")
                nc.scalar.copy(rT[:], tp[:])
                outp = ps_o.tile([64, 53], f32, tag="outp")
                for hh in range(2):
                    for l in range(2):
                        nc.tensor.matmul(
                            outp[:],
                            rT[:, hh * 128 + l * 64 : hh * 128 + l * 64 + 64],
                            dt4[:, (hh * 2 + l) * 53 : (hh * 2 + l + 1) * 53],
                            start=(hh == 0 and l == 0),
                            stop=(hh == 1 and l == 1),
                        )
                outs = miscp.tile([64, 53], f32, tag="outs")
                nc.vector.tensor_tensor(outs[:], outp[:], bb[:], AO.add)
                nc.sync.dma_start(out=d_out[k], in_=outs[:])

            def flush_tail():
                if pending[0] is not None:
                    k, t, u2, a2, xh = pending[0]
                    nc.tensor.matmul(
                        u2[:], a2[:], xh[:, 0:257],
                        start=(t == 0), stop=(t == Tc - 1),
                    )
                    if t == Tc - 1:
                        epilogue(k, u2)
                    pending[0] = None

            u2 = None
            for k in range(CHUNKS_PER_CORE):
                for t in range(Tc):
                    g = k * Tc + t
                    xt = xtp.tile([128, 256], f16, tag="xt")
                    xh = xhp.tile([128, 257], f16, tag="xh")
                    sgl = sgp.tile([128, 2], f32, tag="sgl")
                    nc.sync.dma_start(out=xt[:], in_=d_xt[g])
                    nc.sync.dma_start(out=xh[:], in_=d_xh[g])
                    nc.sync.dma_start(out=sgl[:], in_=d_sg[g])

                    if t == 0:
                        u2 = ps_u.tile([128, 257], f32, tag="u2")
                    fl = ps_fl.tile([128, CW], f32, tag="fl")
                    nc.tensor.matmul(
                        fl[:], xt[:, 0:128], ct0[:], start=True, stop=False
                    )
                    nc.tensor.matmul(
                        fl[:], xt[:, 128:256], ct1[:], start=False, stop=True
                    )
                    flush_tail()

                    E = epool.tile([128, CW], f16, tag="E")
                    nc.scalar.activation(E[:], fl[:], EXP)
                    et = etpool.tile([128, 2], f32, tag="et")
                    sj = jpool.tile([128, CW], f16, tag="sj")
                    # sgl cols: 0=label, 1=seg-rel; both class blocks
                    # share the label onehot (L0 gathered by class on host)
                    nc.vector.scalar_tensor_tensor(
                        sj[:, 0:56], io56[:], sgl[:, 0:1], E[:, 0:56],
                        AO.is_equal, AO.mult, accum_out=et[:, 0:1],
                    )
                    nc.vector.scalar_tensor_tensor(
                        sj[:, 56:112], io56[:], sgl[:, 0:1], E[:, 56:112],
                        AO.is_equal, AO.mult, accum_out=et[:, 1:2],
                    )
                    a2 = a2pool.tile([128, 128], f16, tag="a2")
                    nc.vector.tensor_scalar(
                        a2[:, 0:64], io64[:], sgl[:, 1:2], et[:, 0:1],
                        AO.is_equal, AO.mult,
                    )
                    nc.vector.tensor_scalar(
                        a2[:, 64:128], io64[:], sgl[:, 1:2], et[:, 1:2],
                        AO.is_equal, AO.mult,
                    )
                    pending[0] = (k, t, u2, a2, xh)
            flush_tail()

    _split_all_waits(nc)
    return nc


def _prep(x, rel_emb0, rel_emb1, disc, bias, relation_levels, label_index, scope):
    seg = _segment_ids(np.asarray(scope))
    counts = np.bincount(seg, minlength=N_BAGS).astype(np.int64)
    cum = np.concatenate([[0], np.cumsum(counts)])
    assign, max_load = _lpt_chunks(counts)
    Tc = max(1, (max_load + 127) // 128)
    G = CHUNKS_PER_CORE * Tc
    f16 = np.float16

    x = np.asarray(x, np.float32)
    rl = np.asarray(relation_levels, np.int64)
    labels = np.asarray(label_index, np.float32)

    ctT = np.zeros((256, CW), np.float32)
    ctT[:, 0:53] = np.asarray(rel_emb0, np.float32)[rl[:, 0]].T   # L0 by class
    ctT[:, 56:109] = np.asarray(rel_emb1, np.float32)[rl[:, 1]].T  # L1 by class
    ct = np.stack([ctT[0:128], ctT[128:256]], 0).astype(f16)

    io56 = np.full((56,), 1000.0, np.float32)
    io56[0:53] = np.arange(53)
    io56 = np.broadcast_to(io56, (128, 56)).astype(f16)
    io64 = np.broadcast_to(
        np.arange(64, dtype=np.float32), (128, 64)
    ).astype(f16)
    identb = np.eye(128, dtype=np.float32)
    disc = np.asarray(disc, np.float32)
    dt4 = np.zeros((128, 4, 53), np.float32)
    for hh in range(2):
        for l in range(2):
            dt4[:, hh * 2 + l, :] = disc[:, l * 256 + hh * 128 :
                                         l * 256 + (hh + 1) * 128].T
    dt4 = dt4.reshape(128, 212).astype(f16)
    biasb = np.broadcast_to(
        np.asarray(bias, np.float32), (64, 53)
    ).copy()

    const = {
        "ct": ct, "io56": io56, "io64": io64,
        "identb": identb, "dt4": dt4, "biasb": biasb,
    }

    in_maps = []
    meta = []
    for core in range(NCORE):
        xt3 = np.zeros((G, 128, 256), f16)
        xh3 = np.zeros((G, 128, 257), f16)
        sg3 = np.zeros((G, 128, 2), np.float32)
        sg3[:, :, 1] = -1.0  # seg-rel pad: no bag match
        cmeta = []
        for kk in range(CHUNKS_PER_CORE):
            bags = assign[core * CHUNKS_PER_CORE + kk]
            if bags:
                sents = np.concatenate(
                    [np.arange(cum[b], cum[b + 1]) for b in bags]
                )
                srel = np.repeat(
                    np.arange(len(bags), dtype=np.float32),
                    counts[np.asarray(bags)],
                )
            else:
                sents = np.zeros((0,), np.int64)
                srel = np.zeros((0,), np.float32)
            L = len(sents)
            sl = slice(kk * Tc, (kk + 1) * Tc)
            Xc = np.zeros((Tc * 128, 256), np.float32)
            Xc[0:L] = x[sents]
            xt3[sl] = (
                Xc.reshape(Tc, 128, 2, 128)
                .transpose(0, 3, 2, 1)
                .reshape(Tc, 128, 256)
                .astype(f16)
            )
            xh3[sl, :, 0:256] = Xc.astype(f16).reshape(Tc, 128, 256)
            col = np.zeros((Tc * 128,), np.float32)
            col[0:L] = 1.0
            xh3[sl, :, 256] = col.astype(f16).reshape(Tc, 128)
            col = np.zeros((Tc * 128,), np.float32)
            col[0:L] = labels[sents]
            sg3[sl, :, 0] = col.reshape(Tc, 128)
            col = np.full((Tc * 128,), -1.0, np.float32)
            col[0:L] = srel
            sg3[sl, :, 1] = col.reshape(Tc, 128)
            cmeta.append(bags)
        meta.append(cmeta)
        in_maps.append({"xt3": xt3, "xh3": xh3, "sgl3": sg3, **const})
    return Tc, in_maps, meta


def kernel(x, rel_emb0, rel_emb1, disc, bias, relation_levels, label_index,
           scope, _trace=False):
    from concourse.bass_utils import run_bass_kernel_spmd

    Tc, in_maps, meta = _prep(
        x, rel_emb0, rel_emb1, disc, bias, relation_levels, label_index, scope
    )
    if Tc not in _CACHE:
        _CACHE[Tc] = _build_bass(Tc)
    nc = _CACHE[Tc]
    res = None
    for attempt in range(3):
        try:
            res = run_bass_kernel_spmd(
                nc, in_maps, core_ids=list(range(NCORE)), trace=_trace
            )
            break
        except Exception:
            if attempt == 2:
                raise
    out = np.zeros((N_BAGS, NCLS), np.float32)
    for core in range(NCORE):
        o = np.asarray(res.results[core]["out"])
        for kk, bags in enumerate(meta[core]):
            if bags:
                out[np.asarray(bags)] = o[kk, 0 : len(bags)]
    kernel._last_results = res
    return out
